# revision 3
# baseline (speedup 1.0000x reference)
"""CrossAttentionFusion forward on 8 Trainium2 NeuronCores (pure data parallel).

Math folded on host (seq-len-1 MHA == two chained linears):
  d_att = micro @ A_dm + c_dm,  A_dm = Wv_dm.T @ Wout_dm.T
  m_att = drug  @ A_md + c_md
  u = drug + d_att ; w = micro + m_att
  xu = (u - mu)/sd ; xw likewise        (LN affine folded into W1)
  h1 = gelu([xu, xw] @ W1f + b1f),  W1f = (ffn_w1 * g_cat).T
  h2 = h1 @ W2f + b2,               W2f = ffn_w2.T
  out = ((h2 - mu)/sd) * g_out + b_out

Device layout: activations feature-major [feat(partition), batch(free)];
batch sharded across 8 cores, tiles of NB=512 columns.

LN strategy (v2):
  - per-column sums s=-mu and s2=E[x^2] via ones-matmuls, col-group packed
    (2 concurrent chains per PSUM bank at output partitions 0/32/64/96)
  - small-vector chain on ACT+DVE produces bf16 [negmu, inv] staging rows
  - staging rows bounce through an Internal DRAM tensor and come back as a
    partition-broadcast DMA ([1,N] -> [128,N]), so the normalize runs on DVE
    with all-SBUF bf16 operands (2x/4x DVE modes) and no PE broadcast matmuls
  - gelu merged across pairs of FFN1 m-blocks (one ACT call per 2 PSUM banks)
  - output stored bf16 (host converts to fp32)
All matmuls bf16 with fp32 PSUM accumulation.
"""

import sys

if "/opt/trn_rl_repo" not in sys.path:
    sys.path.insert(0, "/opt/trn_rl_repo")

from contextlib import ExitStack

import ml_dtypes
import numpy as np

import concourse.bass as bass  # noqa: F401  (registers mybir lowering hooks)
import concourse.tile as tile
from concourse import bacc, mybir
from concourse.bass import ts
from concourse.bass_utils import run_bass_kernel_spmd

F32 = mybir.dt.float32
BF16 = mybir.dt.bfloat16
ACT = mybir.ActivationFunctionType
ALU = mybir.AluOpType

P = 128
D = 384
KD = D // P          # 3
DH = 2 * D           # 768
KH = DH // P         # 6
DF = 4 * D           # 1536
KF = DF // P         # 12
EPS = 1e-5
N_CORES = 8
B_FULL = 65536
BC = B_FULL // N_CORES   # 8192 rows per core
NB = 512                 # batch columns per on-chip tile

_NC_CACHE = {}
LAST_RESULTS = None      # BassKernelResults of the most recent kernel() call


def _build_nc(bc, nb, flags):
    use_c_dm, use_c_md, use_b1, use_b2, use_affine = flags
    nt = bc // nb
    nc = bacc.Bacc("TRN2", target_bir_lowering=False, debug=False,
                   num_devices=N_CORES)

    xd_d = nc.dram_tensor("xd", [D, bc], BF16, kind="ExternalInput")
    xm_d = nc.dram_tensor("xm", [D, bc], BF16, kind="ExternalInput")
    a_dm_d = nc.dram_tensor("a_dm", [D, D], BF16, kind="ExternalInput")
    a_md_d = nc.dram_tensor("a_md", [D, D], BF16, kind="ExternalInput")
    w1_d = nc.dram_tensor("w1", [DH, DF], BF16, kind="ExternalInput")
    w2_d = nc.dram_tensor("w2", [DF, D], BF16, kind="ExternalInput")
    c_dm_d = nc.dram_tensor("c_dm", [D], F32, kind="ExternalInput") if use_c_dm else None
    c_md_d = nc.dram_tensor("c_md", [D], F32, kind="ExternalInput") if use_c_md else None
    b1_d = nc.dram_tensor("b1", [DF], F32, kind="ExternalInput") if use_b1 else None
    b2_d = nc.dram_tensor("b2", [D], F32, kind="ExternalInput") if use_b2 else None
    g_o_d = nc.dram_tensor("g_o", [D], F32, kind="ExternalInput") if use_affine else None
    b_o_d = nc.dram_tensor("b_o", [D], F32, kind="ExternalInput") if use_affine else None
    o_d = nc.dram_tensor("o", [D, bc], BF16, kind="ExternalOutput")
    # staging for LN stat vectors: per tile [negmu_u, inv_u, negmu_w, inv_w,
    # negmu_o, inv_o] rows, bounced to DRAM and broadcast-read back.
    stg_d = nc.dram_tensor("stg", [nt, 6, NB], BF16, kind="Internal")

    xd_r = xd_d.ap().rearrange("(k p) n -> p k n", p=P)
    xm_r = xm_d.ap().rearrange("(k p) n -> p k n", p=P)
    o_r = o_d.ap().rearrange("(k p) n -> p k n", p=P)
    stg_r = stg_d.ap()

    with tile.TileContext(nc) as tc, ExitStack() as ctx:
        wp = ctx.enter_context(tc.tile_pool(name="wts", bufs=1))
        xp = ctx.enter_context(tc.tile_pool(name="x", bufs=3))
        up = ctx.enter_context(tc.tile_pool(name="u", bufs=2))
        sqp = ctx.enter_context(tc.tile_pool(name="sq", bufs=2))
        xhp = ctx.enter_context(tc.tile_pool(name="xh", bufs=2))
        h1p = ctx.enter_context(tc.tile_pool(name="h1", bufs=2))
        h2p = ctx.enter_context(tc.tile_pool(name="h2", bufs=2))
        op_ = ctx.enter_context(tc.tile_pool(name="o", bufs=2))
        smp = ctx.enter_context(tc.tile_pool(name="sm", bufs=2))
        bcp = ctx.enter_context(tc.tile_pool(name="bc", bufs=2))
        pmm = ctx.enter_context(tc.tile_pool(name="pmm", bufs=2, space="PSUM"))
        pff = ctx.enter_context(tc.tile_pool(name="pff", bufs=2, space="PSUM"))
        pst = ctx.enter_context(tc.tile_pool(name="pst", bufs=1, space="PSUM"))

        a_dm_sb = wp.tile([P, KD, D], BF16)
        nc.gpsimd.dma_start(a_dm_sb[:], a_dm_d.ap().rearrange("(k p) m -> p k m", p=P))
        a_md_sb = wp.tile([P, KD, D], BF16)
        nc.gpsimd.dma_start(a_md_sb[:], a_md_d.ap().rearrange("(k p) m -> p k m", p=P))
        w1_sb = wp.tile([P, KH, DF], BF16)
        nc.gpsimd.dma_start(w1_sb[:], w1_d.ap().rearrange("(k p) m -> p k m", p=P))
        w2_sb = wp.tile([P, KF, D], BF16)
        nc.gpsimd.dma_start(w2_sb[:], w2_d.ap().rearrange("(k p) m -> p k m", p=P))

        ones_p1 = wp.tile([P, 1], BF16)
        nc.vector.memset(ones_p1[:], 1.0)
        eps_sb = wp.tile([1, 1], F32)
        nc.vector.memset(eps_sb[:], EPS)

        def vec_const(dram, nk, tag):
            t = wp.tile([P, nk], F32, tag=tag)
            nc.gpsimd.dma_start(t[:], dram.ap().rearrange("(k p) -> p k", p=P))
            return t

        c_dm_sb = vec_const(c_dm_d, KD, "c_dm") if use_c_dm else None
        c_md_sb = vec_const(c_md_d, KD, "c_md") if use_c_md else None
        b1_sb = vec_const(b1_d, KF, "b1") if use_b1 else None
        b2_sb = vec_const(b2_d, KD, "b2") if use_b2 else None
        g_o_sb = vec_const(g_o_d, KD, "g_o") if use_affine else None
        b_o_sb = vec_const(b_o_d, KD, "b_o") if use_affine else None

        def stat_chain(st, p_s, p_s2, stage, slot, tag):
            """From packed sums (s=sum(x) at psum partition p_s, s2=sum(x^2)
            at p_s2) produce stage[:, slot]=negmu16, stage[:, slot+1]=inv16."""
            # negmu16 = -(1/D) * s   (exact fp32 scale on ACT)
            nc.scalar.activation(stage[:, slot, :], st[p_s:p_s + 1, :],
                                 ACT.Copy, scale=float(-1.0 / D))
            mu2 = smp.tile([1, NB], F32, tag=f"mu2{tag}")
            nc.vector.tensor_mul(mu2[:], stage[:, slot, :], stage[:, slot, :])
            # var = (1/D)*s2 - mu^2
            var = smp.tile([1, NB], F32, tag=f"var{tag}")
            nc.vector.scalar_tensor_tensor(var[:], st[p_s2:p_s2 + 1, :],
                                           float(1.0 / D), mu2[:],
                                           ALU.mult, ALU.subtract)
            sd = smp.tile([1, NB], F32, tag=f"sd{tag}")
            nc.scalar.activation(sd[:], var[:], ACT.Sqrt, bias=eps_sb[:])
            inv = smp.tile([1, NB], F32, tag=f"inv{tag}")
            nc.vector.reciprocal_approx_fast(inv[:], sd[:])
            nc.scalar.activation(stage[:, slot + 1, :], inv[:], ACT.Copy)

        for it in range(nt):
            sl = slice(it * NB, (it + 1) * NB)
            xd = xp.tile([P, KD, NB], BF16, tag="xd")
            nc.sync.dma_start(xd[:], xd_r[:, :, sl])
            xm = xp.tile([P, KD, NB], BF16, tag="xm")
            nc.sync.dma_start(xm[:], xm_r[:, :, sl])

            def attn(a_sb, rhs, res, c_sb, tag):
                v = up.tile([P, KD, NB], BF16, tag=tag)
                for m in range(KD):
                    ps = pmm.tile([P, NB], F32, tag="mm")
                    for k in range(KD):
                        nc.tensor.matmul(ps[:],
                                         a_sb[:, k, ts(m, P)],
                                         rhs[:, k, :],
                                         start=(k == 0), stop=(k == KD - 1))
                    nc.vector.tensor_add(v[:, m, :], ps[:], res[:, m, :])
                    if c_sb is not None:
                        nc.vector.tensor_scalar_add(v[:, m, :], v[:, m, :],
                                                    c_sb[:, m:m + 1])
                return v

            u = attn(a_dm_sb, xm, xd, c_dm_sb, "u")
            squ = sqp.tile([P, KD, NB], BF16, tag="squ")
            nc.vector.tensor_mul(squ[:], u[:], u[:])
            w = attn(a_md_sb, xd, xm, c_md_sb, "w")
            sqw = sqp.tile([P, KD, NB], BF16, tag="sqw")
            nc.vector.tensor_mul(sqw[:], w[:], w[:])

            # packed stat sums: [s_u@0, s2_u@32, s_w@64, s2_w@96] in one bank
            st = pst.tile([P, NB], F32, tag="uw")
            for k in range(KD):
                nc.tensor.matmul(st[0:1, :], ones_p1[:], u[:, k, :],
                                 start=(k == 0), stop=(k == KD - 1),
                                 tile_position=(0, 0))
                nc.tensor.matmul(st[32:33, :], ones_p1[:], squ[:, k, :],
                                 start=(k == 0), stop=(k == KD - 1),
                                 tile_position=(0, 32))
            for k in range(KD):
                nc.tensor.matmul(st[64:65, :], ones_p1[:], w[:, k, :],
                                 start=(k == 0), stop=(k == KD - 1),
                                 tile_position=(0, 64))
                nc.tensor.matmul(st[96:97, :], ones_p1[:], sqw[:, k, :],
                                 start=(k == 0), stop=(k == KD - 1),
                                 tile_position=(0, 96))

            stage_uw = smp.tile([1, 4, NB], BF16, tag="stage_uw")
            stat_chain(st, 0, 32, stage_uw, 0, "u")
            stat_chain(st, 64, 96, stage_uw, 2, "w")
            nc.sync.dma_start(stg_r[it:it + 1, 0:4, :], stage_uw[:])
            bcuw = bcp.tile([P, 4, NB], BF16, tag="bcuw")
            nc.sync.dma_start(
                bcuw[:], stg_r[it:it + 1, 0:4, :].to_broadcast((P, 4, NB)))

            xhu = xhp.tile([P, KD, NB], BF16, tag="xhu")
            nc.vector.tensor_add(xhu[:], u[:],
                                 bcuw[:, 0:1, :].to_broadcast((P, KD, NB)))
            nc.vector.tensor_mul(xhu[:], xhu[:],
                                 bcuw[:, 1:2, :].to_broadcast((P, KD, NB)))
            xhw = xhp.tile([P, KD, NB], BF16, tag="xhw")
            nc.vector.tensor_add(xhw[:], w[:],
                                 bcuw[:, 2:3, :].to_broadcast((P, KD, NB)))
            nc.vector.tensor_mul(xhw[:], xhw[:],
                                 bcuw[:, 3:4, :].to_broadcast((P, KD, NB)))

            h1 = h1p.tile([P, KF, NB], BF16, tag="h1")
            for g in range(KF // 2):
                ps = pff.tile([P, 2, NB], F32, tag="ff")
                for half in range(2):
                    m = 2 * g + half
                    for k in range(KH):
                        rhs = xhu[:, k, :] if k < KD else xhw[:, k - KD, :]
                        nc.tensor.matmul(ps[:, half, :], w1_sb[:, k, ts(m, P)],
                                         rhs, start=(k == 0), stop=(k == KH - 1))
                if use_b1:
                    for half in range(2):
                        m = 2 * g + half
                        nc.scalar.activation(h1[:, m, :], ps[:, half, :],
                                             ACT.Gelu, bias=b1_sb[:, m:m + 1])
                else:
                    nc.scalar.activation(h1[:, 2 * g:2 * g + 2, :], ps[:],
                                         ACT.Gelu)

            h2 = h2p.tile([P, KD, NB], BF16, tag="h2")
            for m in range(KD):
                ps = pmm.tile([P, NB], F32, tag="mm")
                for k in range(KF):
                    nc.tensor.matmul(ps[:], w2_sb[:, k, ts(m, P)], h1[:, k, :],
                                     start=(k == 0), stop=(k == KF - 1))
                if use_b2:
                    nc.vector.tensor_scalar_add(h2[:, m, :], ps[:],
                                                b2_sb[:, m:m + 1])
                else:
                    nc.vector.tensor_copy(h2[:, m, :], ps[:])
            sqh = sqp.tile([P, KD, NB], BF16, tag="sqh")
            nc.vector.tensor_mul(sqh[:], h2[:], h2[:])

            sto = pst.tile([P, NB], F32, tag="o")
            for k in range(KD):
                nc.tensor.matmul(sto[0:1, :], ones_p1[:], h2[:, k, :],
                                 start=(k == 0), stop=(k == KD - 1))
                nc.tensor.matmul(sto[32:33, :], ones_p1[:], sqh[:, k, :],
                                 start=(k == 0), stop=(k == KD - 1))

            stage_o = smp.tile([1, 2, NB], BF16, tag="stage_o")
            stat_chain(sto, 0, 32, stage_o, 0, "o")
            nc.sync.dma_start(stg_r[it:it + 1, 4:6, :], stage_o[:])
            bco = bcp.tile([P, 2, NB], BF16, tag="bco")
            nc.sync.dma_start(
                bco[:], stg_r[it:it + 1, 4:6, :].to_broadcast((P, 2, NB)))

            o = op_.tile([P, KD, NB], BF16, tag="o")
            nc.vector.tensor_add(o[:], h2[:],
                                 bco[:, 0:1, :].to_broadcast((P, KD, NB)))
            nc.vector.tensor_mul(o[:], o[:],
                                 bco[:, 1:2, :].to_broadcast((P, KD, NB)))
            if use_affine:
                for k in range(KD):
                    nc.vector.tensor_scalar(o[:, k, :], o[:, k, :],
                                            g_o_sb[:, k:k + 1],
                                            b_o_sb[:, k:k + 1],
                                            ALU.mult, ALU.add)
            nc.sync.dma_start(o_r[:, :, sl], o[:])

    nc.compile()
    return nc


def kernel(**inputs) -> np.ndarray:
    global LAST_RESULTS
    f = lambda k: np.asarray(inputs[k], np.float32)

    drug = f("drug_emb")
    micro = f("micro_emb")
    b = drug.shape[0]
    bc = b // N_CORES
    assert b % (N_CORES * NB) == 0

    # ---- host-side weight folding ----
    wv_dm, bv_dm = f("dm_in_w")[2 * D:], f("dm_in_b")[2 * D:]
    wv_md, bv_md = f("md_in_w")[2 * D:], f("md_in_b")[2 * D:]
    a_dm = np.ascontiguousarray(wv_dm.T @ f("dm_out_w").T).astype(ml_dtypes.bfloat16)
    c_dm = bv_dm @ f("dm_out_w").T + f("dm_out_b")
    a_md = np.ascontiguousarray(wv_md.T @ f("md_out_w").T).astype(ml_dtypes.bfloat16)
    c_md = bv_md @ f("md_out_w").T + f("md_out_b")
    g_cat = np.concatenate([f("norm_d_g"), f("norm_m_g")])
    b_cat = np.concatenate([f("norm_d_b"), f("norm_m_b")])
    w1f = np.ascontiguousarray((f("ffn_w1") * g_cat[None, :]).T).astype(ml_dtypes.bfloat16)
    b1f = f("ffn_b1") + b_cat @ f("ffn_w1").T
    w2f = np.ascontiguousarray(f("ffn_w2").T).astype(ml_dtypes.bfloat16)
    b2 = f("ffn_b2")
    g_o, b_o = f("norm_out_g"), f("norm_out_b")

    flags = (bool(np.any(c_dm)), bool(np.any(c_md)), bool(np.any(b1f)),
             bool(np.any(b2)), bool(np.any(g_o != 1.0) or np.any(b_o)))

    key = (bc, NB, flags)
    if key not in _NC_CACHE:
        _NC_CACHE[key] = _build_nc(bc, NB, flags)
    nc = _NC_CACHE[key]

    in_maps = []
    for c in range(N_CORES):
        sl = slice(c * bc, (c + 1) * bc)
        m = {
            "xd": np.ascontiguousarray(drug[sl].T).astype(ml_dtypes.bfloat16),
            "xm": np.ascontiguousarray(micro[sl].T).astype(ml_dtypes.bfloat16),
            "a_dm": a_dm, "a_md": a_md, "w1": w1f, "w2": w2f,
        }
        if flags[0]:
            m["c_dm"] = c_dm
        if flags[1]:
            m["c_md"] = c_md
        if flags[2]:
            m["b1"] = b1f
        if flags[3]:
            m["b2"] = b2
        if flags[4]:
            m["g_o"] = g_o
            m["b_o"] = b_o
        in_maps.append(m)

    res = run_bass_kernel_spmd(nc, in_maps, list(range(N_CORES)))
    LAST_RESULTS = res

    out = np.empty((b, D), np.float32)
    for c in range(N_CORES):
        out[c * bc:(c + 1) * bc] = res.results[c]["o"].T.astype(np.float32)
    return out


# revision 9
# speedup vs baseline: 1.4725x; 1.4725x over previous
"""CrossAttentionFusion forward on 8 Trainium2 NeuronCores (pure data parallel).

Math folded on host (seq-len-1 MHA == two chained linears):
  d_att = micro @ A_dm + c_dm,  A_dm = Wv_dm.T @ Wout_dm.T
  m_att = drug  @ A_md + c_md
  u = drug + d_att ; w = micro + m_att
  xu = (u - mu)/sd ; xw likewise        (LN affine folded into W1)
  h1 = gelu([xu, xw] @ W1f + b1f),  W1f = (ffn_w1 * g_cat).T
  h2 = h1 @ W2f + b2,               W2f = ffn_w2.T
  out = ((h2 - mu)/sd) * g_out + b_out

Device layout: activations feature-major [feat(partition), batch(free)];
batch sharded across 8 cores, tiles of NB=512 columns.

LN strategy (v2):
  - per-column sums s=-mu and s2=E[x^2] via ones-matmuls, col-group packed
    (2 concurrent chains per PSUM bank at output partitions 0/32/64/96)
  - small-vector chain on ACT+DVE produces bf16 [negmu, inv] staging rows
  - staging rows bounce through an Internal DRAM tensor and come back as a
    partition-broadcast DMA ([1,N] -> [128,N]), so the normalize runs on DVE
    with all-SBUF bf16 operands (2x/4x DVE modes) and no PE broadcast matmuls
  - gelu merged across pairs of FFN1 m-blocks (one ACT call per 2 PSUM banks)
  - output stored bf16 (host converts to fp32)
All matmuls bf16 with fp32 PSUM accumulation.
"""

import sys

if "/opt/trn_rl_repo" not in sys.path:
    sys.path.insert(0, "/opt/trn_rl_repo")

from contextlib import ExitStack

import ml_dtypes
import numpy as np

import concourse.bass as bass  # noqa: F401  (registers mybir lowering hooks)
import concourse.tile as tile
from concourse import bacc, mybir
from concourse.bass import ts
from concourse.bass_utils import run_bass_kernel_spmd

F32 = mybir.dt.float32
BF16 = mybir.dt.bfloat16
ACT = mybir.ActivationFunctionType
ALU = mybir.AluOpType

P = 128
D = 384
KD = D // P          # 3
DH = 2 * D           # 768
KH = DH // P         # 6
DF = 4 * D           # 1536
KF = DF // P         # 12
EPS = 1e-5
N_CORES = 8
B_FULL = 65536
BC = B_FULL // N_CORES   # 8192 rows per core
NB = 512                 # batch columns per on-chip tile

_NC_CACHE = {}
LAST_RESULTS = None      # BassKernelResults of the most recent kernel() call


def _build_nc(bc, nb, flags):
    use_c_dm, use_c_md, use_b1, use_b2, use_affine = flags
    nt = bc // nb
    nc = bacc.Bacc("TRN2", target_bir_lowering=False, debug=False,
                   num_devices=N_CORES)

    xd_d = nc.dram_tensor("xd", [D, bc], BF16, kind="ExternalInput")
    xm_d = nc.dram_tensor("xm", [D, bc], BF16, kind="ExternalInput")
    a_dm_d = nc.dram_tensor("a_dm", [D, D], BF16, kind="ExternalInput")
    a_md_d = nc.dram_tensor("a_md", [D, D], BF16, kind="ExternalInput")
    w1_d = nc.dram_tensor("w1", [DH, DF], BF16, kind="ExternalInput")
    w2_d = nc.dram_tensor("w2", [DF, D], BF16, kind="ExternalInput")
    c_dm_d = nc.dram_tensor("c_dm", [D], F32, kind="ExternalInput") if use_c_dm else None
    c_md_d = nc.dram_tensor("c_md", [D], F32, kind="ExternalInput") if use_c_md else None
    b1_d = nc.dram_tensor("b1", [DF], F32, kind="ExternalInput") if use_b1 else None
    b2_d = nc.dram_tensor("b2", [D], F32, kind="ExternalInput") if use_b2 else None
    g_o_d = nc.dram_tensor("g_o", [D], F32, kind="ExternalInput") if use_affine else None
    b_o_d = nc.dram_tensor("b_o", [D], F32, kind="ExternalInput") if use_affine else None
    o_d = nc.dram_tensor("o", [D, bc], BF16, kind="ExternalOutput")
    # staging for LN stat vectors: per tile [negmu_u, inv_u, negmu_w, inv_w,
    # negmu_o, inv_o] rows, bounced to DRAM and broadcast-read back.
    stg_d = nc.dram_tensor("stg", [nt, 6, NB], BF16, kind="Internal")

    xd_r = xd_d.ap().rearrange("(k p) n -> p k n", p=P)
    xm_r = xm_d.ap().rearrange("(k p) n -> p k n", p=P)
    o_r = o_d.ap().rearrange("(k p) n -> p k n", p=P)
    stg_r = stg_d.ap()

    with tile.TileContext(nc) as tc, ExitStack() as ctx:
        wp = ctx.enter_context(tc.tile_pool(name="wts", bufs=1))
        xp = ctx.enter_context(tc.tile_pool(name="x", bufs=3))
        up = ctx.enter_context(tc.tile_pool(name="u", bufs=3))
        sqp = ctx.enter_context(tc.tile_pool(name="sq", bufs=2))
        xhp = ctx.enter_context(tc.tile_pool(name="xh", bufs=3))
        h1p = ctx.enter_context(tc.tile_pool(name="h1", bufs=2))
        h2p = ctx.enter_context(tc.tile_pool(name="h2", bufs=2))
        op_ = ctx.enter_context(tc.tile_pool(name="o", bufs=2))
        smp = ctx.enter_context(tc.tile_pool(name="sm", bufs=2))
        bcp = ctx.enter_context(tc.tile_pool(name="bc", bufs=2))
        # PSUM bank budget (8): attn ring 2 + ffn1 ring 2 + ffn2 ring 2
        # + stats uw 1 + stats o 1. Separate rings per stage so the
        # scheduler can run tile t+1's attention while tile t's LN chain
        # (ACT/DVE/DMA) is in flight.
        pmm = ctx.enter_context(tc.tile_pool(name="pmm", bufs=2, space="PSUM"))
        pff = ctx.enter_context(tc.tile_pool(name="pff", bufs=2, space="PSUM"))
        pst = ctx.enter_context(tc.tile_pool(name="pst", bufs=1, space="PSUM"))

        a_dm_sb = wp.tile([P, KD, D], BF16)
        nc.gpsimd.dma_start(a_dm_sb[:], a_dm_d.ap().rearrange("(k p) m -> p k m", p=P))
        a_md_sb = wp.tile([P, KD, D], BF16)
        nc.gpsimd.dma_start(a_md_sb[:], a_md_d.ap().rearrange("(k p) m -> p k m", p=P))
        w1_sb = wp.tile([P, KH, DF], BF16)
        nc.gpsimd.dma_start(w1_sb[:], w1_d.ap().rearrange("(k p) m -> p k m", p=P))
        w2_sb = wp.tile([P, KF, D], BF16)
        nc.gpsimd.dma_start(w2_sb[:], w2_d.ap().rearrange("(k p) m -> p k m", p=P))

        ones_p1 = wp.tile([P, 1], BF16)
        nc.vector.memset(ones_p1[:], 1.0)
        eps_sb = wp.tile([1, 1], F32)
        nc.vector.memset(eps_sb[:], EPS)

        def vec_const(dram, nk, tag):
            t = wp.tile([P, nk], F32, tag=tag)
            nc.gpsimd.dma_start(t[:], dram.ap().rearrange("(k p) -> p k", p=P))
            return t

        c_dm_sb = vec_const(c_dm_d, KD, "c_dm") if use_c_dm else None
        c_md_sb = vec_const(c_md_d, KD, "c_md") if use_c_md else None
        b1_sb = vec_const(b1_d, KF, "b1") if use_b1 else None
        b2_sb = vec_const(b2_d, KD, "b2") if use_b2 else None
        g_o_sb = vec_const(g_o_d, KD, "g_o") if use_affine else None
        b_o_sb = vec_const(b_o_d, KD, "b_o") if use_affine else None

        def stat_chain(st, p_s, p_s2, stage, slot, tag):
            """From packed sums (s=sum(x) at psum partition p_s, s2=sum(x^2)
            at p_s2) produce stage[:, slot]=negmu16, stage[:, slot+1]=inv16."""
            # negmu16 = -(1/D) * s   (exact fp32 scale on ACT)
            nc.scalar.activation(stage[:, slot, :], st[p_s:p_s + 1, :],
                                 ACT.Copy, scale=float(-1.0 / D))
            scr = smp.tile([1, 2, NB], F32, tag=f"scr{tag}")
            # scr0 = mu^2 ; scr1 = var = (1/D)*s2 - mu^2 ; scr0 = sd ; scr1 = 1/sd
            nc.vector.tensor_mul(scr[:, 0, :], stage[:, slot, :],
                                 stage[:, slot, :])
            nc.vector.scalar_tensor_tensor(scr[:, 1, :], st[p_s2:p_s2 + 1, :],
                                           float(1.0 / D), scr[:, 0, :],
                                           ALU.mult, ALU.subtract)
            nc.scalar.activation(scr[:, 0, :], scr[:, 1, :], ACT.Sqrt,
                                 bias=eps_sb[:])
            nc.vector.reciprocal_approx_fast(scr[:, 1, :], scr[:, 0, :])
            nc.scalar.activation(stage[:, slot + 1, :], scr[:, 1, :], ACT.Copy)

        for it in range(nt):
            sl = slice(it * NB, (it + 1) * NB)
            xd = xp.tile([P, KD, NB], BF16, tag="xd")
            nc.sync.dma_start(xd[:], xd_r[:, :, sl])
            xm = xp.tile([P, KD, NB], BF16, tag="xm")
            nc.sync.dma_start(xm[:], xm_r[:, :, sl])

            def attn(a_sb, rhs, res, c_sb, tag):
                v = up.tile([P, KD, NB], BF16, tag=tag)
                for m in range(KD):
                    ps = pmm.tile([P, NB], F32, tag="att")
                    for k in range(KD):
                        nc.tensor.matmul(ps[:],
                                         a_sb[:, k, ts(m, P)],
                                         rhs[:, k, :],
                                         start=(k == 0), stop=(k == KD - 1))
                    nc.vector.tensor_add(v[:, m, :], ps[:], res[:, m, :])
                    if c_sb is not None:
                        nc.vector.tensor_scalar_add(v[:, m, :], v[:, m, :],
                                                    c_sb[:, m:m + 1])
                return v

            u = attn(a_dm_sb, xm, xd, c_dm_sb, "u")
            squ = sqp.tile([P, KD, NB], BF16, tag="squ")
            nc.vector.tensor_mul(squ[:], u[:], u[:])
            w = attn(a_md_sb, xd, xm, c_md_sb, "w")
            sqw = sqp.tile([P, KD, NB], BF16, tag="sqw")
            nc.vector.tensor_mul(sqw[:], w[:], w[:])

            # packed stat sums: [s_u@0, s2_u@32, s_w@64, s2_w@96] in one bank
            st = pst.tile([P, NB], F32, tag="uw")
            for k in range(KD):
                nc.tensor.matmul(st[0:1, :], ones_p1[:], u[:, k, :],
                                 start=(k == 0), stop=(k == KD - 1),
                                 tile_position=(0, 0))
                nc.tensor.matmul(st[32:33, :], ones_p1[:], squ[:, k, :],
                                 start=(k == 0), stop=(k == KD - 1),
                                 tile_position=(0, 32))
            for k in range(KD):
                nc.tensor.matmul(st[64:65, :], ones_p1[:], w[:, k, :],
                                 start=(k == 0), stop=(k == KD - 1),
                                 tile_position=(0, 64))
                nc.tensor.matmul(st[96:97, :], ones_p1[:], sqw[:, k, :],
                                 start=(k == 0), stop=(k == KD - 1),
                                 tile_position=(0, 96))

            stage_uw = smp.tile([1, 4, NB], BF16, tag="stage_uw")
            stat_chain(st, 0, 32, stage_uw, 0, "u")
            stat_chain(st, 64, 96, stage_uw, 2, "w")
            nc.sync.dma_start(stg_r[it:it + 1, 0:4, :], stage_uw[:])
            bcuw = bcp.tile([P, 4, NB], BF16, tag="bcuw")
            nc.sync.dma_start(
                bcuw[:], stg_r[it:it + 1, 0:4, :].to_broadcast((P, 4, NB)))

            xhu = xhp.tile([P, KD, NB], BF16, tag="xhu")
            nc.vector.tensor_add(xhu[:], u[:],
                                 bcuw[:, 0:1, :].to_broadcast((P, KD, NB)))
            nc.vector.tensor_mul(xhu[:], xhu[:],
                                 bcuw[:, 1:2, :].to_broadcast((P, KD, NB)))
            xhw = xhp.tile([P, KD, NB], BF16, tag="xhw")
            nc.vector.tensor_add(xhw[:], w[:],
                                 bcuw[:, 2:3, :].to_broadcast((P, KD, NB)))
            nc.vector.tensor_mul(xhw[:], xhw[:],
                                 bcuw[:, 3:4, :].to_broadcast((P, KD, NB)))

            h1 = h1p.tile([P, KF, NB], BF16, tag="h1")
            for m in range(KF):
                ps = pff.tile([P, NB], F32, tag="ff")
                for k in range(KH):
                    rhs = xhu[:, k, :] if k < KD else xhw[:, k - KD, :]
                    nc.tensor.matmul(ps[:], w1_sb[:, k, ts(m, P)],
                                     rhs, start=(k == 0), stop=(k == KH - 1))
                if use_b1:
                    nc.scalar.activation(h1[:, m, :], ps[:], ACT.Gelu,
                                         bias=b1_sb[:, m:m + 1])
                else:
                    nc.scalar.activation(h1[:, m, :], ps[:], ACT.Gelu)

            h2 = h2p.tile([P, KD, NB], BF16, tag="h2")
            for m in range(KD):
                ps = pmm.tile([P, NB], F32, tag="f2")
                for k in range(KF):
                    nc.tensor.matmul(ps[:], w2_sb[:, k, ts(m, P)], h1[:, k, :],
                                     start=(k == 0), stop=(k == KF - 1))
                if use_b2:
                    nc.vector.tensor_scalar_add(h2[:, m, :], ps[:],
                                                b2_sb[:, m:m + 1])
                else:
                    nc.vector.tensor_copy(h2[:, m, :], ps[:])
            sqh = sqp.tile([P, KD, NB], BF16, tag="sqh")
            nc.vector.tensor_mul(sqh[:], h2[:], h2[:])

            sto = pst.tile([P, NB], F32, tag="so")
            for k in range(KD):
                nc.tensor.matmul(sto[0:1, :], ones_p1[:], h2[:, k, :],
                                 start=(k == 0), stop=(k == KD - 1))
                nc.tensor.matmul(sto[32:33, :], ones_p1[:], sqh[:, k, :],
                                 start=(k == 0), stop=(k == KD - 1))

            stage_o = smp.tile([1, 2, NB], BF16, tag="stage_o")
            stat_chain(sto, 0, 32, stage_o, 0, "o")
            nc.sync.dma_start(stg_r[it:it + 1, 4:6, :], stage_o[:])
            bco = bcp.tile([P, 2, NB], BF16, tag="bco")
            nc.sync.dma_start(
                bco[:], stg_r[it:it + 1, 4:6, :].to_broadcast((P, 2, NB)))

            o = op_.tile([P, KD, NB], BF16, tag="o")
            nc.vector.tensor_add(o[:], h2[:],
                                 bco[:, 0:1, :].to_broadcast((P, KD, NB)))
            nc.vector.tensor_mul(o[:], o[:],
                                 bco[:, 1:2, :].to_broadcast((P, KD, NB)))
            if use_affine:
                for k in range(KD):
                    nc.vector.tensor_scalar(o[:, k, :], o[:, k, :],
                                            g_o_sb[:, k:k + 1],
                                            b_o_sb[:, k:k + 1],
                                            ALU.mult, ALU.add)
            nc.sync.dma_start(o_r[:, :, sl], o[:])

    nc.compile()
    return nc


def kernel(**inputs) -> np.ndarray:
    global LAST_RESULTS
    f = lambda k: np.asarray(inputs[k], np.float32)

    drug = f("drug_emb")
    micro = f("micro_emb")
    b = drug.shape[0]
    bc = b // N_CORES
    assert b % (N_CORES * NB) == 0

    # ---- host-side weight folding ----
    wv_dm, bv_dm = f("dm_in_w")[2 * D:], f("dm_in_b")[2 * D:]
    wv_md, bv_md = f("md_in_w")[2 * D:], f("md_in_b")[2 * D:]
    a_dm = np.ascontiguousarray(wv_dm.T @ f("dm_out_w").T).astype(ml_dtypes.bfloat16)
    c_dm = bv_dm @ f("dm_out_w").T + f("dm_out_b")
    a_md = np.ascontiguousarray(wv_md.T @ f("md_out_w").T).astype(ml_dtypes.bfloat16)
    c_md = bv_md @ f("md_out_w").T + f("md_out_b")
    g_cat = np.concatenate([f("norm_d_g"), f("norm_m_g")])
    b_cat = np.concatenate([f("norm_d_b"), f("norm_m_b")])
    w1f = np.ascontiguousarray((f("ffn_w1") * g_cat[None, :]).T).astype(ml_dtypes.bfloat16)
    b1f = f("ffn_b1") + b_cat @ f("ffn_w1").T
    w2f = np.ascontiguousarray(f("ffn_w2").T).astype(ml_dtypes.bfloat16)
    b2 = f("ffn_b2")
    g_o, b_o = f("norm_out_g"), f("norm_out_b")

    flags = (bool(np.any(c_dm)), bool(np.any(c_md)), bool(np.any(b1f)),
             bool(np.any(b2)), bool(np.any(g_o != 1.0) or np.any(b_o)))

    key = (bc, NB, flags)
    if key not in _NC_CACHE:
        _NC_CACHE[key] = _build_nc(bc, NB, flags)
    nc = _NC_CACHE[key]

    in_maps = []
    for c in range(N_CORES):
        sl = slice(c * bc, (c + 1) * bc)
        m = {
            "xd": np.ascontiguousarray(drug[sl].T).astype(ml_dtypes.bfloat16),
            "xm": np.ascontiguousarray(micro[sl].T).astype(ml_dtypes.bfloat16),
            "a_dm": a_dm, "a_md": a_md, "w1": w1f, "w2": w2f,
        }
        if flags[0]:
            m["c_dm"] = c_dm
        if flags[1]:
            m["c_md"] = c_md
        if flags[2]:
            m["b1"] = b1f
        if flags[3]:
            m["b2"] = b2
        if flags[4]:
            m["g_o"] = g_o
            m["b_o"] = b_o
        in_maps.append(m)

    res = run_bass_kernel_spmd(nc, in_maps, list(range(N_CORES)))
    LAST_RESULTS = res

    out = np.empty((b, D), np.float32)
    for c in range(N_CORES):
        out[c * bc:(c + 1) * bc] = res.results[c]["o"].T.astype(np.float32)
    return out
